# revision 25
# baseline (speedup 1.0000x reference)
"""Conv2D 3x3 (stride 1, pad 1) Trainium2 Bass kernel.

Problem: x (32, 64, 64, 64) NCHW fp32, weight (128, 64, 3, 3) OIHW, bias (128,).
Output: (32, 128, 64, 64).

Strategy: data-parallel over batch across 8 cores (4 images/core). The host
pre-pads each image channel into a 66x66 zero-ringed layout (+ tail slack) and
pre-rounds x/weights to the PE's fp32r grid (1s+8e+11m, round-to-nearest).
On-chip, partitions 0-63 hold the padded channels and partitions 64-127 hold
the same data shifted down one padded row (a second DMA of the same HBM bytes
at offset 66), so a single K=128 matmul contracts two kernel-row taps at once.
Conv = 6 accumulating fp32r matmuls per 384-pixel PSUM tile (3 paired
ky={0,1} + 3 single ky=2); fp32r runs at full PE rate for moving dim >= 256.
Bias-add fuses into the PSUM->SBUF eviction on the scalar engine.
"""

import numpy as np

import concourse.bass as bass
import concourse.mybir as mybir
import concourse.tile as tile
from concourse import bacc
from concourse.bass_utils import run_bass_kernel_spmd
from concourse.tile_rust import add_dep_helper

N_CORES = 8
NIMG = 4  # images per core
C = 64  # input channels
H = W = 64
O = 128  # output channels
PW = 66  # padded row length
PH = 66  # padded rows
IMG = PH * PW  # 4356 padded elements per channel per image
QTOT = H * PW  # 4224 output positions in padded indexing (64 rows x 66)
QT = 384  # pixels per PSUM tile (must divide QTOT, >=256 for f32r full rate)
NQT = QTOT // QT  # 11
TAIL = 134  # slack so shifted reads stay in-bounds
XCOLS = IMG + TAIL  # 4490
UPLEN = QTOT + 8  # 4232: columns needed in the shifted upper half

F32 = mybir.dt.float32
F32R = mybir.dt.float32r

_CACHED_NC = None


def build_nc():
    nc = bacc.Bacc()
    x_in = nc.declare_dram_parameter("xp", [NIMG, C, XCOLS], F32R, isOutput=False)
    w_in = nc.declare_dram_parameter("wcat", [2 * C, 6, O], F32R, isOutput=False)
    b_in = nc.declare_dram_parameter("bias", [O, 1], F32, isOutput=False)
    out = nc.declare_dram_parameter("out", [NIMG, O, H, W], F32, isOutput=True)

    with tile.TileContext(nc) as tc:
        with (
            tc.tile_pool(name="const", bufs=1) as const_pool,
            tc.tile_pool(name="xp", bufs=3) as x_pool,
            tc.tile_pool(name="osb", bufs=2) as o_pool,
            tc.tile_pool(name="psum0", bufs=4, space="PSUM") as psum0_pool,
            tc.tile_pool(name="psum", bufs=4, space="PSUM") as psum_pool,
        ):
            wcat = const_pool.tile([2 * C, 6, O], F32R)
            bias_t = const_pool.tile([O, 1], F32)
            wcat_dma = nc.sync.dma_start(wcat[:, :, :], w_in[:, :, :])
            nc.sync.dma_start(bias_t[:, :], b_in[:, :])

            # Dummy 1x1 matmul reading only wcat: absorbs the weight-DMA
            # wait so the first real matmul carries a single sync wait (the
            # fused fp32r LDWEIGHTS+MM instruction has one wait slot).
            # (fp32r ISA: innermost free counts must be even, dst partition 0)
            warm = psum_pool.tile([2, 2], F32, tag="acc")
            warm_mm = nc.tensor.matmul(
                warm[:, :], wcat[0:1, 0, 0:2], wcat[0:1, 0, 0:2],
                start=True, stop=True,
            )

            for m in range(NIMG):
                xt = x_pool.tile([128, XCOLS], F32R)
                # lower half: padded image; upper half: same shifted one
                # padded row (pairs kernel rows ky=0/1 in one K=128 matmul)
                nc.sync.dma_start(xt[0:C, :], x_in[m, :, :])
                nc.sync.dma_start(
                    xt[C : 2 * C, 0:UPLEN], x_in[m, :, PW : PW + UPLEN]
                )

                osb = o_pool.tile([O, QTOT], F32)
                for t in range(NQT):
                    q0 = QT * t
                    pool = psum0_pool if t == 0 else psum_pool
                    acc = pool.tile([O, QT], F32, tag="acc")
                    # ky=2 singles first: they read only the lower xt half,
                    # keeping per-matmul semaphore waits within the fused
                    # fp32r LDWEIGHTS+MM wait-slot budget.
                    for kx in range(3):
                        mm = nc.tensor.matmul(
                            acc[:, :],
                            wcat[0:C, 3 + kx, :],
                            xt[0:C, q0 + 2 * PW + kx : q0 + 2 * PW + kx + QT],
                            start=(kx == 0),
                            stop=False,
                        )
                        if m == 0 and t == 0 and kx == 0:
                            add_dep_helper(
                                mm.ins, warm_mm.ins, sync=False, reason="warm first"
                            )
                    for kx in range(3):
                        nc.tensor.matmul(
                            acc[:, :],
                            wcat[:, kx, :],
                            xt[0 : 2 * C, q0 + kx : q0 + kx + QT],
                            start=False,
                            stop=(kx == 2),
                        )
                    # evict + bias add on the scalar engine
                    nc.scalar.activation(
                        osb[:, q0 : q0 + QT],
                        acc[:, :],
                        mybir.ActivationFunctionType.Identity,
                        bias=bias_t[:, :],
                    )

                # store the 64 valid columns of each output row
                ov = osb[:, :].rearrange("p (y c) -> p y c", c=PW)
                nc.sync.dma_start(out[m, :, :, :], ov[:, :, 0:W])

    nc.compile()
    return nc


def _round_fp32r(a: np.ndarray) -> np.ndarray:
    """Round fp32 to the fp32r grid (11 mantissa bits, RNE)."""
    a = np.ascontiguousarray(a, dtype=np.float32)
    u = a.view(np.uint32)
    low = u & np.uint32(0xFFF)
    lsb = (u >> np.uint32(12)) & np.uint32(1)
    round_up = (low > 0x800) | ((low == 0x800) & (lsb == 1))
    r = (u & np.uint32(0xFFFFF000)) + (round_up.astype(np.uint32) << np.uint32(12))
    return r.view(np.float32)


def _prep_inputs(x, weight, bias):
    x = _round_fp32r(np.asarray(x, dtype=np.float32))
    n = x.shape[0]
    z = np.zeros((n, C, PH, PW), dtype=np.float32)
    z[:, :, 1 : 1 + H, 1 : 1 + W] = x
    xp = np.zeros((n, C, XCOLS), dtype=np.float32)
    xp[:, :, :IMG] = z.reshape(n, C, IMG)

    w_t = _round_fp32r(np.asarray(weight, dtype=np.float32)).transpose(1, 2, 3, 0)
    wcat = np.zeros((2 * C, 6, O), dtype=np.float32)
    wcat[0:C, 0:3, :] = w_t[:, 0, :, :]  # ky=0 (lower half of pairs)
    wcat[C : 2 * C, 0:3, :] = w_t[:, 1, :, :]  # ky=1 (upper half of pairs)
    wcat[0:C, 3:6, :] = w_t[:, 2, :, :]  # ky=2 singles
    b = np.ascontiguousarray(np.asarray(bias, dtype=np.float32).reshape(O, 1))
    return xp, wcat, b


def _in_maps(x, weight, bias):
    xp, wcat, b = _prep_inputs(x, weight, bias)
    return [
        {"xp": xp[i * NIMG : (i + 1) * NIMG], "wcat": wcat, "bias": b}
        for i in range(N_CORES)
    ]


def kernel(x: np.ndarray, weight: np.ndarray, bias: np.ndarray) -> np.ndarray:
    global _CACHED_NC
    if _CACHED_NC is None:
        _CACHED_NC = build_nc()
    res = run_bass_kernel_spmd(_CACHED_NC, _in_maps(x, weight, bias), list(range(N_CORES)))
    return np.concatenate([r["out"] for r in res.results], axis=0)


def run_profiled(x, weight, bias, tmpdir=None):
    """Dev helper: run with NTFF tracing, return BassKernelResults."""
    global _CACHED_NC
    if _CACHED_NC is None:
        _CACHED_NC = build_nc()
    return run_bass_kernel_spmd(
        _CACHED_NC, _in_maps(x, weight, bias), list(range(N_CORES)),
        trace=True, tmpdir=tmpdir,
    )
